# revision 14
# baseline (speedup 1.0000x reference)
"""Trainium2 Bass kernel for nn_M50 (EXP-HYDRO/M50 hydrology ODE + MLP discharge).

The dopri5 reference is matched to ~1.7e-5 relmax by fixed-step RK4 (h=1 day);
that sequential recurrence is solved parallel-in-time: snow via Jacobi-DEER
(linearized affine scan), water via Picard (cumsum scan), with the two 16-wide
MLPs batched over all 999 steps x 4 RK4 stages as chunk-block-diagonal matmuls.
Scans are log-depth doubling; cross-chunk carries via PE transposes.
Raw Bass emission (TileContext sync is broken in this toolchain): every
instruction is chained on a global ticket semaphore pair (compute +1, DMA +16),
fully serializing execution. Work is replicated across the 8 cores.
"""
import numpy as np
from contextlib import ExitStack

T = 1000
C = 8
L = 125
DF, TMAX, TMIN = 2.674, 0.176, -2.093
SNOW_ITERS = 5
WATER_ITERS = 3
HC = (0.5, 0.5, 1.0)
CLAMP = 6.0

_cache = {}


def _build_program():
    import concourse.bass as bass
    import concourse.mybir as mybir

    f32 = mybir.dt.float32
    AL = mybir.AluOpType
    AF = mybir.ActivationFunctionType
    nc = bass.Bass()

    x_d = nc.dram_tensor("x", [T, 3], f32, kind="ExternalInput")
    pr_d = nc.dram_tensor("precp", [T], f32, kind="ExternalInput")
    tm_d = nc.dram_tensor("temp", [T], f32, kind="ExternalInput")
    ld_d = nc.dram_tensor("lday", [T], f32, kind="ExternalInput")
    w1et_d = [nc.dram_tensor(f"w1et{i}", [8, 128], f32, kind="ExternalInput") for i in range(3)]
    w1q_d = [nc.dram_tensor(f"w1q{i}", [8, 128], f32, kind="ExternalInput") for i in range(2)]
    w2et_d = nc.dram_tensor("w2et", [128, 128], f32, kind="ExternalInput")
    w3et_d = nc.dram_tensor("w3et", [128, 8], f32, kind="ExternalInput")
    w2q_d = nc.dram_tensor("w2q", [128, 128], f32, kind="ExternalInput")
    w3q_d = nc.dram_tensor("w3q", [128, 8], f32, kind="ExternalInput")
    b1et_d = nc.dram_tensor("b1et", [128, 1], f32, kind="ExternalInput")
    b2et_d = nc.dram_tensor("b2et", [128, 1], f32, kind="ExternalInput")
    b1q_d = nc.dram_tensor("b1q", [128, 1], f32, kind="ExternalInput")
    b2q_d = nc.dram_tensor("b2q", [128, 1], f32, kind="ExternalInput")
    b3et_d = nc.dram_tensor("b3et", [8, 1], f32, kind="ExternalInput")
    b3q_d = nc.dram_tensor("b3q", [8, 1], f32, kind="ExternalInput")
    id8_d = nc.dram_tensor("id8", [8, 8], f32, kind="ExternalInput")
    out_d = nc.dram_tensor("out", [T], f32, kind="ExternalOutput")

    OPS = []  # (engine_name, emit_fn)

    class _EngRec:
        def __init__(self, eng):
            self._eng = eng
        def __getattr__(self, op):
            def rec(*a, **k):
                OPS.append((self._eng, lambda: getattr(getattr(nc, self._eng), op)(*a, **k)))
            return rec

    V = _EngRec("vector")
    A_ = _EngRec("scalar")
    P = _EngRec("tensor")
    G = _EngRec("gpsimd")

    stack = ExitStack()
    _ctr = [0]

    def sbuf(shape):
        _ctr[0] += 1
        return stack.enter_context(nc.sbuf_tensor(f"sb{_ctr[0]}", shape, f32))

    def psum(shape):
        _ctr[0] += 1
        return stack.enter_context(nc.psum_tensor(f"ps{_ctr[0]}", shape, f32))

    # ---- SBUF/PSUM allocations ----
    x_sb = sbuf([C, 375])
    w1et = [sbuf([8, 128]) for _ in range(3)]
    w1q = [sbuf([8, 128]) for _ in range(2)]
    w2et = sbuf([128, 128]); w3et = sbuf([128, 8])
    w2q = sbuf([128, 128]); w3q = sbuf([128, 8])
    b1et = sbuf([128, 1]); b2et = sbuf([128, 1])
    b1q = sbuf([128, 1]); b2q = sbuf([128, 1])
    b3et = sbuf([8, 1]); b3q = sbuf([8, 1]); id8 = sbuf([8, 8])
    nats = {}; sm1s = {}
    for nm in ("pr", "tm", "ld"):
        nats[nm] = sbuf([C, L]); sm1s[nm] = sbuf([C, L])
    ones_l = sbuf([C, L]); ones_r = sbuf([1, 8]); zero_s = sbuf([C, 4 * L])
    zero8 = sbuf([8, 1]); c_tmin_n = sbuf([8, 1]); c_tmin_p = sbuf([8, 1]); c_tmax = sbuf([8, 1])
    PRst = sbuf([C, 4 * L]); TMst = sbuf([C, 4 * L]); LDst = sbuf([C, 4 * L])
    tmp_s = sbuf([C, 4 * L]); Ps_st = sbuf([C, 4 * L]); Pr_st = sbuf([C, 4 * L])
    Am_st = sbuf([C, 4 * L]); Cm_st = sbuf([C, 4 * L])
    s0col = sbuf([8, 2])
    Zs = sbuf([C, 4 * L]); Sday = sbuf([C, L])
    melt = sbuf([C, 4 * L]); dmelt = sbuf([C, 4 * L]); tZ = sbuf([C, 4 * L])
    sig = sbuf([C, 4 * L]); dsg = sbuf([C, 4 * L]); mn = sbuf([C, 4 * L]); ind = sbuf([C, 4 * L])
    Ks = sbuf([C, 4 * L]); Asc = sbuf([C, L]); Bsc = sbuf([C, L])
    t1 = sbuf([C, L]); t2 = sbuf([C, L]); ysc = sbuf([C, L]); Psc = sbuf([C, L])
    ends = sbuf([8, 2]); prow = sbuf([1, 8]); carry = sbuf([1, 8]); ent = sbuf([1, 8])
    scr1 = sbuf([C, L]); scr2 = sbuf([C, L])
    rowA = sbuf([1, 8]); rowY = sbuf([1, 8]); rowT = sbuf([1, 8]); rowU = sbuf([1, 8])
    CONSTw = sbuf([C, 4 * L]); Zw = sbuf([C, 4 * L]); Wday = sbuf([C, L])
    ETc = sbuf([C, 4 * L]); Qc = sbuf([C, 4 * L]); eET = sbuf([C, 4 * L]); eQ = sbuf([C, 4 * L])
    sigw = sbuf([C, 4 * L]); Kw = sbuf([C, 4 * L]); Bw = sbuf([C, L]); yw = sbuf([C, L])
    h1sb = sbuf([128, 4 * L]); h2sb = sbuf([128, 4 * L]); out_sb = sbuf([C, L])
    mmA = psum([128, 4 * L]); mmB = psum([128, 4 * L])
    outE = psum([8, 4 * L]); outQ = psum([8, 4 * L]); smp = psum([8, 8])

    S4 = [slice(s * L, (s + 1) * L) for s in range(4)]

    # ---------- loads ----------
    G.dma_start(x_sb[:], x_d[:].rearrange("(c f) k -> c (f k)", c=C))
    for i in range(3):
        G.dma_start(w1et[i][:], w1et_d[i][:])
    for i in range(2):
        G.dma_start(w1q[i][:], w1q_d[i][:])
    for sb_t, d_t in ((w2et, w2et_d), (w3et, w3et_d), (w2q, w2q_d), (w3q, w3q_d),
                      (b1et, b1et_d), (b2et, b2et_d), (b1q, b1q_d), (b2q, b2q_d),
                      (b3et, b3et_d), (b3q, b3q_d), (id8, id8_d)):
        G.dma_start(sb_t[:], d_t[:])
    for nm, d_t in (("pr", pr_d), ("tm", tm_d), ("ld", ld_d)):
        G.dma_start(nats[nm][:], d_t[:].rearrange("(c f) -> c f", c=C))
        G.dma_start(sm1s[nm][:, 1:L], d_t[:].rearrange("(c f) -> c f", c=C)[:, 0:L - 1])
        G.dma_start(sm1s[nm][1:C, 0:1], d_t[:].rearrange("(c f) -> c f", c=C)[0:C - 1, L - 1:L])
        V.memset(sm1s[nm][0:1, 0:1], 0.0)
    V.memset(ones_l[:], 1.0); V.memset(ones_r[:], 1.0); V.memset(zero_s[:], 0.0)
    V.memset(zero8[:], 0.0)
    V.memset(c_tmin_n[:], 10.0 * TMIN); V.memset(c_tmin_p[:], -10.0 * TMIN)
    V.memset(c_tmax[:], -10.0 * TMAX)

    # ---------- stage forcing ----------
    for st, nm in ((PRst, "pr"), (TMst, "tm"), (LDst, "ld")):
        nat, m1 = nats[nm], sm1s[nm]
        V.tensor_copy(st[:, 0:L], m1[:])
        V.tensor_add(st[:, L:2 * L], m1[:], nat[:])
        V.tensor_scalar_mul(st[:, L:2 * L], st[:, L:2 * L], 0.5)
        V.tensor_copy(st[:, 2 * L:3 * L], st[:, L:2 * L])
        V.tensor_copy(st[:, 3 * L:4 * L], nat[:])
    A_.activation(tmp_s[:], TMst[:], AF.Sigmoid, bias=c_tmin_n[:], scale=-10.0)
    V.tensor_mul(Ps_st[:], tmp_s[:], PRst[:])
    A_.activation(tmp_s[:], TMst[:], AF.Sigmoid, bias=c_tmin_p[:], scale=10.0)
    V.tensor_mul(Pr_st[:], tmp_s[:], PRst[:])
    A_.activation(Am_st[:], TMst[:], AF.Sigmoid, bias=c_tmax[:], scale=10.0)
    V.tensor_scalar(Cm_st[:], TMst[:], TMAX, DF, AL.subtract, AL.mult)

    # ---------- initial broadcasts ----------
    P.matmul(smp[0:8, 0:2], ones_r[:], x_sb[0:1, 0:2])
    V.tensor_copy(s0col[:], smp[0:8, 0:2])
    V.tensor_scalar(Zs[:], zero_s[:], s0col[:, 0:1], None, AL.add)
    V.tensor_scalar(Sday[:], zero_s[:, 0:L], s0col[:, 0:1], None, AL.add)

    def dbl_affine(y, Acur, B, Ain, tmp, tmpA, Lr):
        V.tensor_copy(y[:, 0:Lr], B[:, 0:Lr])
        V.tensor_copy(Acur[:, 0:Lr], Ain[:, 0:Lr])
        s = 1
        while s < Lr:
            w = Lr - s
            V.tensor_mul(tmp[:, 0:w], y[:, 0:w], Acur[:, s:Lr])
            V.tensor_add(y[:, s:Lr], y[:, s:Lr], tmp[:, 0:w])
            V.tensor_mul(tmpA[:, 0:w], Acur[:, 0:w], Acur[:, s:Lr])
            V.tensor_copy(Acur[:, s:Lr], tmpA[:, 0:w])
            s *= 2

    def dbl_cumsum(y, tmp, Lr):
        s = 1
        while s < Lr:
            w = Lr - s
            V.tensor_add(tmp[:, 0:w], y[:, s:Lr], y[:, 0:w])
            V.tensor_copy(y[:, s:Lr], tmp[:, 0:w])
            s *= 2

    def melt_chain(Ztile, want_deriv):
        A_.activation(tZ[:], Ztile[:], AF.Tanh, bias=zero8[:], scale=5.0)
        V.tensor_scalar(sig[:], tZ[:], 0.5, 0.5, AL.mult, AL.add)
        V.tensor_tensor(mn[:], Ztile[:], Cm_st[:], AL.min)
        V.tensor_mul(melt[:], sig[:], mn[:])
        V.tensor_mul(melt[:], melt[:], Am_st[:])
        if want_deriv:
            V.tensor_mul(dsg[:], tZ[:], tZ[:])
            V.tensor_scalar(dsg[:], dsg[:], -2.5, 2.5, AL.mult, AL.add)
            V.tensor_tensor(ind[:], Ztile[:], Cm_st[:], AL.is_lt)
            V.tensor_mul(dsg[:], dsg[:], mn[:])
            V.tensor_mul(ind[:], ind[:], sig[:])
            V.tensor_add(dmelt[:], dsg[:], ind[:])
            V.tensor_mul(dmelt[:], dmelt[:], Am_st[:])

    def rk_combine(Ktile, out, scale_):
        V.tensor_add(t1[:], Ktile[:, S4[0]], Ktile[:, S4[3]])
        V.tensor_add(t2[:], Ktile[:, S4[1]], Ktile[:, S4[2]])
        V.scalar_tensor_tensor(t1[:], t2[:], 2.0, t1[:], AL.mult, AL.add)
        V.tensor_scalar_mul(out, t1[:], scale_)

    def chunk_carry(y, init_ap, Ptile=None):
        V.tensor_copy(ends[:, 0:1], y[:, L - 1:L])
        P.transpose(smp[0:1, 0:8], ends[:, 0:1], id8[:])
        V.tensor_copy(rowY[:], smp[0:1, 0:8])
        if Ptile is not None:
            V.tensor_copy(ends[:, 1:2], Ptile[:, L - 1:L])
            P.transpose(smp[0:1, 0:8], ends[:, 1:2], id8[:])
            V.tensor_copy(prow[:], smp[0:1, 0:8])
        else:
            V.tensor_copy(prow[:], ones_r[:])
        dbl_affine(rowT, rowA, rowY, prow, rowU, carry, 8)
        V.scalar_tensor_tensor(carry[:], rowA[:], init_ap, rowT[:], AL.mult, AL.add)
        V.tensor_copy(ent[:, 0:1], init_ap)
        V.tensor_copy(ent[:, 1:8], carry[:, 0:7])
        P.matmul(smp[0:8, 0:1], ent[:], ones_r[0:1, 0:1])
        return smp[0:8, 0:1]

    # ================= SNOW =================
    for _ in range(SNOW_ITERS):
        melt_chain(Zs, True)
        V.tensor_sub(Ks[:], Ps_st[:], melt[:])
        rk_combine(dmelt, Asc[:], -1.0 / 6.0)
        V.tensor_scalar(Asc[:], Asc[:], 1.0, 5.0, AL.add, AL.min)
        V.tensor_scalar_max(Asc[:], Asc[:], -5.0)
        rk_combine(Ks, Bsc[:], 1.0 / 6.0)
        V.tensor_mul(t2[:], Asc[:], Sday[:])
        V.tensor_add(Bsc[:], Bsc[:], Sday[:])
        V.tensor_sub(Bsc[:], Bsc[:], t2[:])
        V.memset(Asc[0:1, 0:1], 1.0)
        V.memset(Bsc[0:1, 0:1], 0.0)
        dbl_affine(ysc, Psc, Bsc, Asc, scr1, scr2, L)
        ecol = chunk_carry(ysc, x_sb[0:1, 0:1], Ptile=Psc)
        V.scalar_tensor_tensor(Sday[:], Psc[:], ecol, ysc[:], AL.mult, AL.add)
        V.tensor_copy(Zs[:, S4[0]], Sday[:])
        for j in range(3):
            V.scalar_tensor_tensor(Zs[:, S4[j + 1]], Ks[:, S4[j]], HC[j], Sday[:], AL.mult, AL.add)

    melt_chain(Zs, False)
    V.tensor_add(CONSTw[:], Pr_st[:], melt[:])

    # ================= WATER =================
    V.tensor_scalar(Zw[:], zero_s[:], s0col[:, 1:2], None, AL.add)
    V.tensor_scalar(Wday[:], zero_s[:, 0:L], s0col[:, 1:2], None, AL.add)

    def mlp_eval(feats, w1, w2, w3, b1, b2, out_sb_t, out_ps):
        nf = len(feats)
        for s in range(4):
            for i in range(nf):
                P.matmul(mmA[:, S4[s]], w1[i][:], feats[i][:, S4[s]],
                         start=(i == 0), stop=(i == nf - 1))
        A_.activation(h1sb[:], mmA[:], AF.Tanh, bias=b1[:])
        for s in range(4):
            P.matmul(mmB[:, S4[s]], w2[:], h1sb[:, S4[s]])
        A_.activation(h2sb[:], mmB[:], AF.Tanh, bias=b2[:])
        for s in range(4):
            P.matmul(out_ps[:, S4[s]], w3[:], h2sb[:, S4[s]])
        V.tensor_scalar(out_sb_t[:], out_ps[:], CLAMP, -CLAMP, AL.min, AL.max)

    for _ in range(WATER_ITERS):
        mlp_eval([Zs, Zw, TMst], w1et, w2et, w3et, b1et, b2et, ETc, outE)
        mlp_eval([Zw, PRst], w1q, w2q, w3q, b1q, b2q, Qc, outQ)
        A_.activation(eET[:], ETc[:], AF.Exp, bias=b3et[:])
        A_.activation(eQ[:], Qc[:], AF.Exp, bias=b3q[:])
        V.tensor_mul(eET[:], eET[:], LDst[:])
        V.tensor_add(eET[:], eET[:], eQ[:])
        A_.activation(sigw[:], Zw[:], AF.Sigmoid, bias=zero8[:], scale=10.0)
        V.tensor_mul(sigw[:], sigw[:], eET[:])
        V.tensor_sub(Kw[:], CONSTw[:], sigw[:])
        rk_combine(Kw, Bw[:], 1.0 / 6.0)
        V.memset(Bw[0:1, 0:1], 0.0)
        V.tensor_copy(yw[:], Bw[:])
        dbl_cumsum(yw, scr1, L)
        ecol = chunk_carry(yw, x_sb[0:1, 1:2], Ptile=None)
        V.tensor_scalar(Wday[:], yw[:], ecol, None, AL.add)
        V.tensor_copy(Zw[:, S4[0]], Wday[:])
        for j in range(3):
            V.scalar_tensor_tensor(Zw[:, S4[j + 1]], Kw[:, S4[j]], HC[j], Wday[:], AL.mult, AL.add)
        V.tensor_scalar(Zw[:], Zw[:], 1e4, -1e4, AL.min, AL.max)

    # ================= OUTPUT =================
    P.matmul(mmA[:, 0:L], w1q[0][:], Wday[:], start=True, stop=False)
    P.matmul(mmA[:, 0:L], w1q[1][:], x_sb[:, 2:375:3], start=False, stop=True)
    A_.activation(h1sb[:, 0:L], mmA[:, 0:L], AF.Tanh, bias=b1q[:])
    P.matmul(mmB[:, 0:L], w2q[:], h1sb[:, 0:L])
    A_.activation(h2sb[:, 0:L], mmB[:, 0:L], AF.Tanh, bias=b2q[:])
    P.matmul(outQ[:, 0:L], w3q[:], h2sb[:, 0:L])
    A_.activation(out_sb[:], outQ[:, 0:L], AF.Exp, bias=b3q[:])
    G.dma_start(out_d[:].rearrange("(c f) -> c f", c=C), out_sb[:])

    # ---- emit with global ticket chain ----
    with (nc.semaphore("Tsem") as Tsem, nc.semaphore("Dsem") as Dsem, nc.Block() as block):
        pred = []
        tv, dv = 0, 0
        for eng, _fn in OPS:
            pred.append((Tsem, tv) if pred or True else None)
            # placeholder; recompute below
        pred = []
        cur = None
        for eng, _fn in OPS:
            pred.append(cur)
            if eng == "gpsimd":
                dv += 16
                cur = ("D", dv)
            else:
                tv += 1
                cur = ("T", tv)

        def emit_for(eng_name, eng_obj):
            for i, (eng, fn) in enumerate(OPS):
                if eng != eng_name:
                    continue
                if pred[i] is not None:
                    s, v = pred[i]
                    eng_obj.wait_ge(Tsem if s == "T" else Dsem, v)
                inst = fn()
                if eng_name == "gpsimd":
                    inst.then_inc(Dsem, 16)
                else:
                    inst.then_inc(Tsem, 1)

        @block.vector
        def _(v):
            emit_for("vector", v)

        @block.scalar
        def _(s):
            emit_for("scalar", s)

        @block.tensor
        def _(t):
            emit_for("tensor", t)

        @block.gpsimd
        def _(g):
            with nc.allow_non_contiguous_dma(reason="7-element shifted series tail"):
                emit_for("gpsimd", g)

    stack.close()
    return nc


def _prep_inputs(inputs):
    f32 = np.float32
    x = np.ascontiguousarray(np.asarray(inputs["x"], f32))
    pr = np.ascontiguousarray(np.asarray(inputs["precp_series"], f32))
    tm = np.ascontiguousarray(np.asarray(inputs["temp_series"], f32))
    ld = np.ascontiguousarray(np.asarray(inputs["lday_series"], f32))
    et_p = [np.asarray(a, f32) for a in inputs["et_params"]]
    q_p = [np.asarray(a, f32) for a in inputs["q_params"]]

    def blk1(W, kdim):
        out = np.zeros((kdim * 8, 128), f32)
        for c in range(8):
            for i in range(kdim):
                out[i * 8 + c, 16 * c:16 * c + 16] = W[i]
        return out

    def blk2(W):
        out = np.zeros((128, 128), f32)
        for c in range(8):
            out[16 * c:16 * c + 16, 16 * c:16 * c + 16] = W
        return out

    def blk3(W):
        out = np.zeros((128, 8), f32)
        for c in range(8):
            out[16 * c:16 * c + 16, c] = W[:, 0]
        return out

    def brep(b):
        return np.tile(np.asarray(b, f32), 8).reshape(128, 1)

    W1et, b1et, W2et, b2et, W3et, b3et = et_p
    W1q, b1q, W2q, b2q, W3q, b3q = q_p
    return {
        "x": x, "precp": pr, "temp": tm, "lday": ld,
        **{f"w1et{i}": blk1(W1et, 3)[8 * i:8 * i + 8] for i in range(3)},
        **{f"w1q{i}": blk1(W1q, 2)[8 * i:8 * i + 8] for i in range(2)},
        "w2et": blk2(W2et), "w3et": blk3(W3et),
        "w2q": blk2(W2q), "w3q": blk3(W3q),
        "b1et": brep(b1et), "b2et": brep(b2et),
        "b1q": brep(b1q), "b2q": brep(b2q),
        "b3et": np.full((8, 1), float(b3et[0]), f32),
        "b3q": np.full((8, 1), float(b3q[0]), f32),
        "id8": np.eye(8, dtype=f32),
    }


def kernel(**inputs):
    from concourse.bass_utils import run_bass_kernel_spmd

    if "nc" not in _cache:
        _cache["nc"] = _build_program()
    nc = _cache["nc"]
    in_map = _prep_inputs(inputs)
    res = run_bass_kernel_spmd(nc, [in_map] * 8, list(range(8)))
    return np.asarray(res.results[0]["out"], np.float32).reshape(T, 1)


# revision 16
# speedup vs baseline: 1.0739x; 1.0739x over previous
"""Trainium2 Bass kernel for nn_M50 (EXP-HYDRO/M50 hydrology ODE + MLP discharge).

The dopri5 reference is matched to ~1.7e-5 relmax by fixed-step RK4 (h=1 day);
that sequential recurrence is solved parallel-in-time: snow via Jacobi-DEER
(linearized affine scan), water via Picard (cumsum scan), with the two 16-wide
MLPs batched over all 999 steps x 4 RK4 stages as chunk-block-diagonal matmuls.
Scans are log-depth doubling; cross-chunk carries via PE transposes.
Raw Bass emission (TileContext sync is broken in this toolchain): every
instruction is chained on a global ticket semaphore pair (compute +1, DMA +16),
fully serializing execution. Work is replicated across the 8 cores.
"""
import numpy as np
from contextlib import ExitStack

T = 1000
C = 8
L = 125
DF, TMAX, TMIN = 2.674, 0.176, -2.093
SNOW_ITERS = 3
WATER_ITERS = 2
HC = (0.5, 0.5, 1.0)
CLAMP = 6.0

_cache = {}


def _build_program():
    import concourse.bass as bass
    import concourse.mybir as mybir

    f32 = mybir.dt.float32
    AL = mybir.AluOpType
    AF = mybir.ActivationFunctionType
    nc = bass.Bass()

    x_d = nc.dram_tensor("x", [T, 3], f32, kind="ExternalInput")
    pr_d = nc.dram_tensor("precp", [T], f32, kind="ExternalInput")
    tm_d = nc.dram_tensor("temp", [T], f32, kind="ExternalInput")
    ld_d = nc.dram_tensor("lday", [T], f32, kind="ExternalInput")
    w1et_d = [nc.dram_tensor(f"w1et{i}", [8, 128], f32, kind="ExternalInput") for i in range(3)]
    w1q_d = [nc.dram_tensor(f"w1q{i}", [8, 128], f32, kind="ExternalInput") for i in range(2)]
    w2et_d = nc.dram_tensor("w2et", [128, 128], f32, kind="ExternalInput")
    w3et_d = nc.dram_tensor("w3et", [128, 8], f32, kind="ExternalInput")
    w2q_d = nc.dram_tensor("w2q", [128, 128], f32, kind="ExternalInput")
    w3q_d = nc.dram_tensor("w3q", [128, 8], f32, kind="ExternalInput")
    b1et_d = nc.dram_tensor("b1et", [128, 1], f32, kind="ExternalInput")
    b2et_d = nc.dram_tensor("b2et", [128, 1], f32, kind="ExternalInput")
    b1q_d = nc.dram_tensor("b1q", [128, 1], f32, kind="ExternalInput")
    b2q_d = nc.dram_tensor("b2q", [128, 1], f32, kind="ExternalInput")
    b3et_d = nc.dram_tensor("b3et", [8, 1], f32, kind="ExternalInput")
    b3q_d = nc.dram_tensor("b3q", [8, 1], f32, kind="ExternalInput")
    id8_d = nc.dram_tensor("id8", [8, 8], f32, kind="ExternalInput")
    out_d = nc.dram_tensor("out", [T], f32, kind="ExternalOutput")

    OPS = []  # (engine_name, emit_fn)

    class _EngRec:
        def __init__(self, eng):
            self._eng = eng
        def __getattr__(self, op):
            def rec(*a, **k):
                OPS.append((self._eng, lambda: getattr(getattr(nc, self._eng), op)(*a, **k)))
            return rec

    V = _EngRec("vector")
    A_ = _EngRec("scalar")
    P = _EngRec("tensor")
    G = _EngRec("gpsimd")

    stack = ExitStack()
    _ctr = [0]

    def sbuf(shape):
        _ctr[0] += 1
        return stack.enter_context(nc.sbuf_tensor(f"sb{_ctr[0]}", shape, f32))

    def psum(shape):
        _ctr[0] += 1
        return stack.enter_context(nc.psum_tensor(f"ps{_ctr[0]}", shape, f32))

    # ---- SBUF/PSUM allocations ----
    x_sb = sbuf([C, 375])
    w1et = [sbuf([8, 128]) for _ in range(3)]
    w1q = [sbuf([8, 128]) for _ in range(2)]
    w2et = sbuf([128, 128]); w3et = sbuf([128, 8])
    w2q = sbuf([128, 128]); w3q = sbuf([128, 8])
    b1et = sbuf([128, 1]); b2et = sbuf([128, 1])
    b1q = sbuf([128, 1]); b2q = sbuf([128, 1])
    b3et = sbuf([8, 1]); b3q = sbuf([8, 1]); id8 = sbuf([8, 8])
    nats = {}; sm1s = {}
    for nm in ("pr", "tm", "ld"):
        nats[nm] = sbuf([C, L]); sm1s[nm] = sbuf([C, L])
    ones_l = sbuf([C, L]); ones_r = sbuf([1, 8]); zero_s = sbuf([C, 4 * L])
    zero8 = sbuf([8, 1]); c_tmin_n = sbuf([8, 1]); c_tmin_p = sbuf([8, 1]); c_tmax = sbuf([8, 1])
    PRst = sbuf([C, 4 * L]); TMst = sbuf([C, 4 * L]); LDst = sbuf([C, 4 * L])
    tmp_s = sbuf([C, 4 * L]); Ps_st = sbuf([C, 4 * L]); Pr_st = sbuf([C, 4 * L])
    Am_st = sbuf([C, 4 * L]); Cm_st = sbuf([C, 4 * L])
    s0col = sbuf([8, 2])
    Zs = sbuf([C, 4 * L]); Sday = sbuf([C, L])
    melt = sbuf([C, 4 * L]); dmelt = sbuf([C, 4 * L]); tZ = sbuf([C, 4 * L])
    sig = sbuf([C, 4 * L]); dsg = sbuf([C, 4 * L]); mn = sbuf([C, 4 * L]); ind = sbuf([C, 4 * L])
    Ks = sbuf([C, 4 * L]); Asc = sbuf([C, L]); Bsc = sbuf([C, L])
    t1 = sbuf([C, L]); t2 = sbuf([C, L]); ysc = sbuf([C, L]); Psc = sbuf([C, L])
    ends = sbuf([8, 2]); prow = sbuf([1, 8]); carry = sbuf([1, 8]); ent = sbuf([1, 8])
    scr1 = sbuf([C, L]); scr2 = sbuf([C, L])
    rowA = sbuf([1, 8]); rowY = sbuf([1, 8]); rowT = sbuf([1, 8]); rowU = sbuf([1, 8])
    CONSTw = sbuf([C, 4 * L]); Zw = sbuf([C, 4 * L]); Wday = sbuf([C, L])
    ETc = sbuf([C, 4 * L]); Qc = sbuf([C, 4 * L]); eET = sbuf([C, 4 * L]); eQ = sbuf([C, 4 * L])
    sigw = sbuf([C, 4 * L]); Kw = sbuf([C, 4 * L]); Bw = sbuf([C, L]); yw = sbuf([C, L])
    h1sb = sbuf([128, 4 * L]); h2sb = sbuf([128, 4 * L]); out_sb = sbuf([C, L])
    mmA = psum([128, 4 * L]); mmB = psum([128, 4 * L])
    outE = psum([8, 4 * L]); outQ = psum([8, 4 * L]); smp = psum([8, 8])

    S4 = [slice(s * L, (s + 1) * L) for s in range(4)]

    # ---------- loads ----------
    G.dma_start(x_sb[:], x_d[:].rearrange("(c f) k -> c (f k)", c=C))
    for i in range(3):
        G.dma_start(w1et[i][:], w1et_d[i][:])
    for i in range(2):
        G.dma_start(w1q[i][:], w1q_d[i][:])
    for sb_t, d_t in ((w2et, w2et_d), (w3et, w3et_d), (w2q, w2q_d), (w3q, w3q_d),
                      (b1et, b1et_d), (b2et, b2et_d), (b1q, b1q_d), (b2q, b2q_d),
                      (b3et, b3et_d), (b3q, b3q_d), (id8, id8_d)):
        G.dma_start(sb_t[:], d_t[:])
    for nm, d_t in (("pr", pr_d), ("tm", tm_d), ("ld", ld_d)):
        G.dma_start(nats[nm][:], d_t[:].rearrange("(c f) -> c f", c=C))
        G.dma_start(sm1s[nm][:, 1:L], d_t[:].rearrange("(c f) -> c f", c=C)[:, 0:L - 1])
        G.dma_start(sm1s[nm][1:C, 0:1], d_t[:].rearrange("(c f) -> c f", c=C)[0:C - 1, L - 1:L])
        V.memset(sm1s[nm][0:1, 0:1], 0.0)
    V.memset(ones_l[:], 1.0); V.memset(ones_r[:], 1.0); V.memset(zero_s[:], 0.0)
    V.memset(zero8[:], 0.0)
    V.memset(c_tmin_n[:], 10.0 * TMIN); V.memset(c_tmin_p[:], -10.0 * TMIN)
    V.memset(c_tmax[:], -10.0 * TMAX)

    # ---------- stage forcing ----------
    for st, nm in ((PRst, "pr"), (TMst, "tm"), (LDst, "ld")):
        nat, m1 = nats[nm], sm1s[nm]
        V.tensor_copy(st[:, 0:L], m1[:])
        V.tensor_add(st[:, L:2 * L], m1[:], nat[:])
        V.tensor_scalar_mul(st[:, L:2 * L], st[:, L:2 * L], 0.5)
        V.tensor_copy(st[:, 2 * L:3 * L], st[:, L:2 * L])
        V.tensor_copy(st[:, 3 * L:4 * L], nat[:])
    A_.activation(tmp_s[:], TMst[:], AF.Sigmoid, bias=c_tmin_n[:], scale=-10.0)
    V.tensor_mul(Ps_st[:], tmp_s[:], PRst[:])
    A_.activation(tmp_s[:], TMst[:], AF.Sigmoid, bias=c_tmin_p[:], scale=10.0)
    V.tensor_mul(Pr_st[:], tmp_s[:], PRst[:])
    A_.activation(Am_st[:], TMst[:], AF.Sigmoid, bias=c_tmax[:], scale=10.0)
    V.tensor_scalar(Cm_st[:], TMst[:], TMAX, DF, AL.subtract, AL.mult)

    # ---------- initial broadcasts ----------
    P.matmul(smp[0:8, 0:2], ones_r[:], x_sb[0:1, 0:2])
    V.tensor_copy(s0col[:], smp[0:8, 0:2])
    V.tensor_scalar(Zs[:], zero_s[:], s0col[:, 0:1], None, AL.add)
    V.tensor_scalar(Sday[:], zero_s[:, 0:L], s0col[:, 0:1], None, AL.add)

    def dbl_affine(y, Acur, B, Ain, tmp, tmpA, Lr):
        V.tensor_copy(y[:, 0:Lr], B[:, 0:Lr])
        V.tensor_copy(Acur[:, 0:Lr], Ain[:, 0:Lr])
        s = 1
        while s < Lr:
            w = Lr - s
            V.tensor_mul(tmp[:, 0:w], y[:, 0:w], Acur[:, s:Lr])
            V.tensor_add(y[:, s:Lr], y[:, s:Lr], tmp[:, 0:w])
            V.tensor_mul(tmpA[:, 0:w], Acur[:, 0:w], Acur[:, s:Lr])
            V.tensor_copy(Acur[:, s:Lr], tmpA[:, 0:w])
            s *= 2

    def dbl_cumsum(y, tmp, Lr):
        s = 1
        while s < Lr:
            w = Lr - s
            V.tensor_add(tmp[:, 0:w], y[:, s:Lr], y[:, 0:w])
            V.tensor_copy(y[:, s:Lr], tmp[:, 0:w])
            s *= 2

    def melt_chain(Ztile, want_deriv):
        A_.activation(tZ[:], Ztile[:], AF.Tanh, bias=zero8[:], scale=5.0)
        V.tensor_scalar(sig[:], tZ[:], 0.5, 0.5, AL.mult, AL.add)
        V.tensor_tensor(mn[:], Ztile[:], Cm_st[:], AL.min)
        V.tensor_mul(melt[:], sig[:], mn[:])
        V.tensor_mul(melt[:], melt[:], Am_st[:])
        if want_deriv:
            V.tensor_mul(dsg[:], tZ[:], tZ[:])
            V.tensor_scalar(dsg[:], dsg[:], -2.5, 2.5, AL.mult, AL.add)
            V.tensor_tensor(ind[:], Ztile[:], Cm_st[:], AL.is_lt)
            V.tensor_mul(dsg[:], dsg[:], mn[:])
            V.tensor_mul(ind[:], ind[:], sig[:])
            V.tensor_add(dmelt[:], dsg[:], ind[:])
            V.tensor_mul(dmelt[:], dmelt[:], Am_st[:])

    def rk_combine(Ktile, out, scale_):
        V.tensor_add(t1[:], Ktile[:, S4[0]], Ktile[:, S4[3]])
        V.tensor_add(t2[:], Ktile[:, S4[1]], Ktile[:, S4[2]])
        V.scalar_tensor_tensor(t1[:], t2[:], 2.0, t1[:], AL.mult, AL.add)
        V.tensor_scalar_mul(out, t1[:], scale_)

    def chunk_carry(y, init_ap, Ptile=None):
        V.tensor_copy(ends[:, 0:1], y[:, L - 1:L])
        P.transpose(smp[0:1, 0:8], ends[:, 0:1], id8[:])
        V.tensor_copy(rowY[:], smp[0:1, 0:8])
        if Ptile is not None:
            V.tensor_copy(ends[:, 1:2], Ptile[:, L - 1:L])
            P.transpose(smp[0:1, 0:8], ends[:, 1:2], id8[:])
            V.tensor_copy(prow[:], smp[0:1, 0:8])
        else:
            V.tensor_copy(prow[:], ones_r[:])
        dbl_affine(rowT, rowA, rowY, prow, rowU, carry, 8)
        V.scalar_tensor_tensor(carry[:], rowA[:], init_ap, rowT[:], AL.mult, AL.add)
        V.tensor_copy(ent[:, 0:1], init_ap)
        V.tensor_copy(ent[:, 1:8], carry[:, 0:7])
        P.matmul(smp[0:8, 0:1], ent[:], ones_r[0:1, 0:1])
        return smp[0:8, 0:1]

    # ================= SNOW =================
    for _ in range(SNOW_ITERS):
        melt_chain(Zs, True)
        V.tensor_sub(Ks[:], Ps_st[:], melt[:])
        rk_combine(dmelt, Asc[:], -1.0 / 6.0)
        V.tensor_scalar(Asc[:], Asc[:], 1.0, 5.0, AL.add, AL.min)
        V.tensor_scalar_max(Asc[:], Asc[:], -5.0)
        rk_combine(Ks, Bsc[:], 1.0 / 6.0)
        V.tensor_mul(t2[:], Asc[:], Sday[:])
        V.tensor_add(Bsc[:], Bsc[:], Sday[:])
        V.tensor_sub(Bsc[:], Bsc[:], t2[:])
        V.memset(Asc[0:1, 0:1], 1.0)
        V.memset(Bsc[0:1, 0:1], 0.0)
        dbl_affine(ysc, Psc, Bsc, Asc, scr1, scr2, L)
        ecol = chunk_carry(ysc, x_sb[0:1, 0:1], Ptile=Psc)
        V.scalar_tensor_tensor(Sday[:], Psc[:], ecol, ysc[:], AL.mult, AL.add)
        V.tensor_copy(Zs[:, S4[0]], Sday[:])
        for j in range(3):
            V.scalar_tensor_tensor(Zs[:, S4[j + 1]], Ks[:, S4[j]], HC[j], Sday[:], AL.mult, AL.add)

    melt_chain(Zs, False)
    V.tensor_add(CONSTw[:], Pr_st[:], melt[:])

    # ================= WATER =================
    V.tensor_scalar(Zw[:], zero_s[:], s0col[:, 1:2], None, AL.add)
    V.tensor_scalar(Wday[:], zero_s[:, 0:L], s0col[:, 1:2], None, AL.add)

    def mlp_eval(feats, w1, w2, w3, b1, b2, out_sb_t, out_ps):
        nf = len(feats)
        for s in range(4):
            for i in range(nf):
                P.matmul(mmA[:, S4[s]], w1[i][:], feats[i][:, S4[s]],
                         start=(i == 0), stop=(i == nf - 1))
        A_.activation(h1sb[:], mmA[:], AF.Tanh, bias=b1[:])
        for s in range(4):
            P.matmul(mmB[:, S4[s]], w2[:], h1sb[:, S4[s]])
        A_.activation(h2sb[:], mmB[:], AF.Tanh, bias=b2[:])
        for s in range(4):
            P.matmul(out_ps[:, S4[s]], w3[:], h2sb[:, S4[s]])
        V.tensor_scalar(out_sb_t[:], out_ps[:], CLAMP, -CLAMP, AL.min, AL.max)

    for _ in range(WATER_ITERS):
        mlp_eval([Zs, Zw, TMst], w1et, w2et, w3et, b1et, b2et, ETc, outE)
        mlp_eval([Zw, PRst], w1q, w2q, w3q, b1q, b2q, Qc, outQ)
        A_.activation(eET[:], ETc[:], AF.Exp, bias=b3et[:])
        A_.activation(eQ[:], Qc[:], AF.Exp, bias=b3q[:])
        V.tensor_mul(eET[:], eET[:], LDst[:])
        V.tensor_add(eET[:], eET[:], eQ[:])
        A_.activation(sigw[:], Zw[:], AF.Sigmoid, bias=zero8[:], scale=10.0)
        V.tensor_mul(sigw[:], sigw[:], eET[:])
        V.tensor_sub(Kw[:], CONSTw[:], sigw[:])
        rk_combine(Kw, Bw[:], 1.0 / 6.0)
        V.memset(Bw[0:1, 0:1], 0.0)
        V.tensor_copy(yw[:], Bw[:])
        dbl_cumsum(yw, scr1, L)
        ecol = chunk_carry(yw, x_sb[0:1, 1:2], Ptile=None)
        V.tensor_scalar(Wday[:], yw[:], ecol, None, AL.add)
        V.tensor_copy(Zw[:, S4[0]], Wday[:])
        for j in range(3):
            V.scalar_tensor_tensor(Zw[:, S4[j + 1]], Kw[:, S4[j]], HC[j], Wday[:], AL.mult, AL.add)
        V.tensor_scalar(Zw[:], Zw[:], 1e4, -1e4, AL.min, AL.max)

    # ================= OUTPUT =================
    P.matmul(mmA[:, 0:L], w1q[0][:], Wday[:], start=True, stop=False)
    P.matmul(mmA[:, 0:L], w1q[1][:], x_sb[:, 2:375:3], start=False, stop=True)
    A_.activation(h1sb[:, 0:L], mmA[:, 0:L], AF.Tanh, bias=b1q[:])
    P.matmul(mmB[:, 0:L], w2q[:], h1sb[:, 0:L])
    A_.activation(h2sb[:, 0:L], mmB[:, 0:L], AF.Tanh, bias=b2q[:])
    P.matmul(outQ[:, 0:L], w3q[:], h2sb[:, 0:L])
    A_.activation(out_sb[:], outQ[:, 0:L], AF.Exp, bias=b3q[:])
    G.dma_start(out_d[:].rearrange("(c f) -> c f", c=C), out_sb[:])

    # ---- emit with global ticket chain ----
    with (nc.semaphore("Tsem") as Tsem, nc.semaphore("Dsem") as Dsem, nc.Block() as block):
        pred = []
        tv, dv = 0, 0
        for eng, _fn in OPS:
            pred.append((Tsem, tv) if pred or True else None)
            # placeholder; recompute below
        pred = []
        cur = None
        for eng, _fn in OPS:
            pred.append(cur)
            if eng == "gpsimd":
                dv += 16
                cur = ("D", dv, eng)
            else:
                tv += 1
                cur = ("T", tv, eng)

        def emit_for(eng_name, eng_obj):
            for i, (eng, fn) in enumerate(OPS):
                if eng != eng_name:
                    continue
                if pred[i] is not None:
                    s, v, _ = pred[i]
                    eng_obj.wait_ge(Tsem if s == "T" else Dsem, v)
                inst = fn()
                if eng_name == "gpsimd":
                    inst.then_inc(Dsem, 16)
                else:
                    inst.then_inc(Tsem, 1)

        @block.vector
        def _(v):
            emit_for("vector", v)

        @block.scalar
        def _(s):
            emit_for("scalar", s)

        @block.tensor
        def _(t):
            emit_for("tensor", t)

        @block.gpsimd
        def _(g):
            with nc.allow_non_contiguous_dma(reason="7-element shifted series tail"):
                emit_for("gpsimd", g)

    stack.close()
    return nc


def _prep_inputs(inputs):
    f32 = np.float32
    x = np.ascontiguousarray(np.asarray(inputs["x"], f32))
    pr = np.ascontiguousarray(np.asarray(inputs["precp_series"], f32))
    tm = np.ascontiguousarray(np.asarray(inputs["temp_series"], f32))
    ld = np.ascontiguousarray(np.asarray(inputs["lday_series"], f32))
    et_p = [np.asarray(a, f32) for a in inputs["et_params"]]
    q_p = [np.asarray(a, f32) for a in inputs["q_params"]]

    def blk1(W, kdim):
        out = np.zeros((kdim * 8, 128), f32)
        for c in range(8):
            for i in range(kdim):
                out[i * 8 + c, 16 * c:16 * c + 16] = W[i]
        return out

    def blk2(W):
        out = np.zeros((128, 128), f32)
        for c in range(8):
            out[16 * c:16 * c + 16, 16 * c:16 * c + 16] = W
        return out

    def blk3(W):
        out = np.zeros((128, 8), f32)
        for c in range(8):
            out[16 * c:16 * c + 16, c] = W[:, 0]
        return out

    def brep(b):
        return np.tile(np.asarray(b, f32), 8).reshape(128, 1)

    W1et, b1et, W2et, b2et, W3et, b3et = et_p
    W1q, b1q, W2q, b2q, W3q, b3q = q_p
    return {
        "x": x, "precp": pr, "temp": tm, "lday": ld,
        **{f"w1et{i}": blk1(W1et, 3)[8 * i:8 * i + 8] for i in range(3)},
        **{f"w1q{i}": blk1(W1q, 2)[8 * i:8 * i + 8] for i in range(2)},
        "w2et": blk2(W2et), "w3et": blk3(W3et),
        "w2q": blk2(W2q), "w3q": blk3(W3q),
        "b1et": brep(b1et), "b2et": brep(b2et),
        "b1q": brep(b1q), "b2q": brep(b2q),
        "b3et": np.full((8, 1), float(b3et[0]), f32),
        "b3q": np.full((8, 1), float(b3q[0]), f32),
        "id8": np.eye(8, dtype=f32),
    }


def kernel(**inputs):
    from concourse.bass_utils import run_bass_kernel_spmd

    if "nc" not in _cache:
        _cache["nc"] = _build_program()
    nc = _cache["nc"]
    in_map = _prep_inputs(inputs)
    res = run_bass_kernel_spmd(nc, [in_map] * 8, list(range(8)))
    return np.asarray(res.results[0]["out"], np.float32).reshape(T, 1)
